# revision 2
# baseline (speedup 1.0000x reference)
"""Trainium2 Bass kernel v3: mixed fp16/fp8 rewired-linear GEMM.

Math: out = x @ W'.T + bias with the rewire correction folded exactly into
W' on the host (see kernel.py).  Data-parallel over 8 cores (4096 rows each).

Speed trick: a fraction of the K reduction runs as fp8e4m3 DoubleRow matmuls
at 2x the fp16 PE rate.  For m-tiles with local index m < MT_FP8, k-subtiles
6..7 (columns 768:1024) use one DoubleRow fp8 matmul instead of two fp16
matmuls.  Both precisions accumulate into the same PSUM chain: ALL operands
are pre-scaled by powers of two (x*16, W'*256 -- exact in fp16, optimal
range for e4m3) so every product carries the same 4096x scale, removed
during the fused PSUM evacuation  out = ps * 2^-12 + bias  on the DVE.

Accuracy: fp8 quantization error contributes 3.7e-2 * sqrt(f_eff) where
f_eff = 0.25 * MT_FP8/32.  MT_FP8=24 -> rel err 1.60e-2 (gate is 2e-2),
verified bit-deterministically against the fixed-seed reference inputs.
"""

import sys
import types

import numpy as np
import ml_dtypes

import concourse.bass as bass  # noqa: F401
import concourse.tile as tile
import concourse.mybir as mybir
from concourse import bacc
from concourse.bass_utils import run_bass_kernel_spmd


def _ensure_ntff_hook():
    try:
        import antenv.axon_hooks  # noqa: F401
        return
    except ImportError:
        pass
    mod = types.ModuleType("antenv.axon_hooks")
    _hook = [None]
    mod.set_axon_ntff_profile_hook = lambda h: _hook.__setitem__(0, h)
    mod.get_axon_ntff_profile_hook = lambda: _hook[0]
    sys.modules["antenv.axon_hooks"] = mod
    try:
        import antenv
        antenv.axon_hooks = mod
        from trn_agent_boot.trn_boot import _ntff_profile_via_ctypes
        mod.set_axon_ntff_profile_hook(
            _ntff_profile_via_ctypes('/opt/axon/libaxon_pjrt.so'))
    except Exception:
        pass


_ensure_ntff_hook()

N_CORES = 8
N = 32768
IN_F = 1024
OUT_F = 1024
P = 128
NS = N // N_CORES          # 4096 rows per core
MT = NS // P               # 32 m-tiles per core
MT2 = MT // 2              # 16 double-m-tiles
KO = IN_F // P             # 8 k-subtiles
KO16 = KO - 2              # k-subtiles 0..5 always fp16
OC = 512                   # PSUM free-dim chunk (one fp32 bank)

MT_FP8 = 24                # m-tiles (of 32 per core) using fp8 for k 6..7

X_SCALE = 16.0
W_SCALE = 256.0
OUT_SCALE = 1.0 / (X_SCALE * W_SCALE)

_nc_cache = None


def _build_nc():
    global _nc_cache
    if _nc_cache is not None:
        return _nc_cache

    nc = bacc.Bacc("TRN2", target_bir_lowering=False, debug=False)
    xb_d = nc.dram_tensor("xb", [MT2, P, 2, KO, P], mybir.dt.float16,
                          kind="ExternalInput")
    x8_d = nc.dram_tensor("x8", [MT2, P, 2, 2, P], mybir.dt.float8e4,
                          kind="ExternalInput")
    wt_d = nc.dram_tensor("wt", [KO, P, OUT_F], mybir.dt.float16,
                          kind="ExternalInput")
    w8_d = nc.dram_tensor("w8", [P, 2, OUT_F], mybir.dt.float8e4,
                          kind="ExternalInput")
    bias_d = nc.dram_tensor("bias", [P, OUT_F], mybir.dt.float16,
                            kind="ExternalInput")
    out_d = nc.dram_tensor("out", [NS, OUT_F], mybir.dt.float16,
                           kind="ExternalOutput")

    with tile.TileContext(nc) as tc:
        with (
            tc.tile_pool(name="wpool", bufs=1) as wpool,
            tc.tile_pool(name="xpool", bufs=2) as xpool,
            tc.tile_pool(name="x8pool", bufs=2) as x8pool,
            tc.tile_pool(name="opool", bufs=4) as opool,
            tc.tile_pool(name="pspool", bufs=6, space="PSUM") as pspool,
            tc.tile_pool(name="warmpool", bufs=1) as warmpool,
            tc.tile_pool(name="warmps", bufs=1, space="PSUM") as warmps,
        ):
            # Prelude DMA schedule, ALL on the Sync queue in strict need
            # order (the two hwdge queues share the same 16 DMA engines /
            # ~330GB/s, so splitting only reorders, never adds bandwidth).
            # fp8 m-tiles don't touch W tiles 6..7, so those and bias are
            # deferred (emitted inside the loop so their SP program position
            # follows xt[1]); they arrive long before they're needed.
            xt0 = xpool.tile([P, 2, KO, P], mybir.dt.float16, tag="xt")
            nc.sync.dma_start(xt0[:, 0], xb_d.ap()[0, :, 0])
            w01 = wpool.tile([P, 2, OUT_F], mybir.dt.float16, tag="w01")
            nc.sync.dma_start(w01[:], wt_d.ap()[0:2].transpose([1, 0, 2]))
            w23 = wpool.tile([P, 2, OUT_F], mybir.dt.float16, tag="w23")
            nc.sync.dma_start(w23[:], wt_d.ap()[2:4].transpose([1, 0, 2]))
            w45 = wpool.tile([P, 2, OUT_F], mybir.dt.float16, tag="w45")
            nc.sync.dma_start(w45[:], wt_d.ap()[4:6].transpose([1, 0, 2]))
            x8t0 = x8pool.tile([P, 2, 2, P], mybir.dt.float8e4, tag="x8t")
            nc.sync.dma_start(x8t0[:], x8_d.ap()[0])
            w8_sb = wpool.tile([P, 2, OUT_F], mybir.dt.float8e4, tag="w8")
            nc.sync.dma_start(w8_sb[:], w8_d.ap())
            nc.sync.dma_start(xt0[:, 1], xb_d.ap()[0, :, 1])
            # Software-pipeline the second super-tile and the late-needed
            # tiles here so emission order == dependency order (bias is
            # first read at m=0's evacuation, w67 first at m-tile MT_FP8;
            # both trail xt[1] in the Sync queue but arrive in time).
            xt1 = xpool.tile([P, 2, KO, P], mybir.dt.float16, tag="xt")
            nc.sync.dma_start(xt1[:], xb_d.ap()[1])
            x8t1 = x8pool.tile([P, 2, 2, P], mybir.dt.float8e4, tag="x8t")
            nc.sync.dma_start(x8t1[:], x8_d.ap()[1])
            bias_sb = wpool.tile([P, OUT_F], mybir.dt.float16, tag="bias")
            nc.sync.dma_start(bias_sb[:], bias_d.ap())
            w67 = wpool.tile([P, 2, OUT_F], mybir.dt.float16, tag="w67")
            nc.sync.dma_start(w67[:], wt_d.ap()[6:8].transpose([1, 0, 2]))

            def wslice(ko, osl):
                if ko < 2:
                    return w01[:, ko, osl]
                if ko < 4:
                    return w23[:, ko - 2, osl]
                if ko < 6:
                    return w45[:, ko - 4, osl]
                return w67[:, ko - 6, osl]

            # PE warm-up on a zeroed tile while the input DMAs stream in
            # (clock ramp: 0.65 -> 2.4 GHz needs ~3us of PE activity); sized
            # to end right as the first m-tile's operands land.
            wrm = warmpool.tile([P, P], mybir.dt.float16, tag="wrm")
            nc.gpsimd.memset(wrm[:], 0.0)
            wps = warmps.tile([P, P], mybir.dt.float32, tag="wps")
            for _ in range(40):
                nc.tensor.matmul(wps[:], wrm[:], wrm[:], start=True, stop=True)

            NOC = OUT_F // OC
            for m2 in range(MT2):
                if m2 == 0:
                    xt, x8t = xt0, x8t0
                elif m2 == 1:
                    xt, x8t = xt1, x8t1
                else:
                    xt = xpool.tile([P, 2, KO, P], mybir.dt.float16, tag="xt")
                    nc.sync.dma_start(xt[:], xb_d.ap()[m2])
                    x8t = x8pool.tile([P, 2, 2, P], mybir.dt.float8e4,
                                      tag="x8t")
                    nc.sync.dma_start(x8t[:], x8_d.ap()[m2])

                for h in range(2):
                    m = 2 * m2 + h
                    use8 = m < MT_FP8
                    out_sb = opool.tile([P, OUT_F], mybir.dt.float16,
                                        tag="osb")
                    pss = [pspool.tile([P, OC], mybir.dt.float32, tag="ps",
                                       name=f"ps{m}_{oc}")
                           for oc in range(NOC)]
                    nko16 = KO16 if use8 else KO
                    for ko in range(nko16):
                        for oc in range(NOC):
                            nc.tensor.matmul(
                                pss[oc][:],
                                xt[:, h, ko, :],
                                wslice(ko, slice(oc * OC, (oc + 1) * OC)),
                                start=(ko == 0),
                                stop=(not use8 and ko == KO - 1),
                            )
                    if use8:
                        for oc in range(NOC):
                            nc.tensor.matmul(
                                pss[oc][:],
                                x8t[:, h, :, :],
                                w8_sb[:, :, oc * OC:(oc + 1) * OC],
                                start=False,
                                stop=True,
                                perf_mode=mybir.MatmulPerfMode.DoubleRow,
                            )
                    for oc in range(NOC):
                        nc.vector.scalar_tensor_tensor(
                            out_sb[:, oc * OC:(oc + 1) * OC], pss[oc][:],
                            OUT_SCALE,
                            bias_sb[:, oc * OC:(oc + 1) * OC],
                            mybir.AluOpType.mult, mybir.AluOpType.add,
                        )
                        if m >= MT - 4:
                            # Tail: store each PSUM half as soon as it's
                            # evacuated, alternating DMA queues.
                            eng = nc.scalar if (m * NOC + oc) % 2 else nc.sync
                            eng.dma_start(
                                out_d.ap()[m * P:(m + 1) * P,
                                           oc * OC:(oc + 1) * OC],
                                out_sb[:, oc * OC:(oc + 1) * OC])
                    if m < MT - 4:
                        nc.scalar.dma_start(out_d.ap()[m * P:(m + 1) * P, :],
                                            out_sb[:])

    nc.compile()
    _nc_cache = nc
    return nc


def _fold_rewires(weight, rewire_rows, rewire_src, rewire_clones):
    """Fold the rewire corrections into the weight matrix (exact, fp32)."""
    r = np.asarray(rewire_rows, dtype=np.int64)
    s = np.asarray(rewire_src, dtype=np.int64)
    d = np.asarray(rewire_clones, dtype=np.int64)
    denom = d.shape[1] + 1
    w_rs = weight[r, s]                      # [R]
    w_rd = weight[r[:, None], d]             # [R, K]
    dW = np.zeros_like(weight)
    np.add.at(dW, (r, s), (1.0 / denom - 1.0) * w_rs + w_rd.sum(axis=1) / denom)
    np.add.at(dW, (r[:, None], d), -w_rd)
    return weight + dW


def _pack_inputs(x, weight, bias, rewire_rows, rewire_src, rewire_clones):
    e4 = ml_dtypes.float8_e4m3
    wp = _fold_rewires(np.asarray(weight, dtype=np.float32),
                       rewire_rows, rewire_src, rewire_clones)
    wps = np.ascontiguousarray(wp.T) * W_SCALE   # [k_g, o], scaled
    wt = wps.astype(np.float16).reshape(KO, P, OUT_F)
    # fp8 W for k-subtiles 6..7: [k, i, o]
    w8 = np.ascontiguousarray(
        wps[KO16 * P:].astype(e4).reshape(2, P, OUT_F).transpose(1, 0, 2))
    bias128 = np.ascontiguousarray(
        np.broadcast_to(np.asarray(bias, dtype=np.float32).astype(np.float16),
                        (P, OUT_F)))

    xs_all = np.asarray(x, dtype=np.float32) * X_SCALE
    xb16 = xs_all.astype(np.float16)
    x8_all = xs_all[:, KO16 * P:].astype(e4)     # [N, 256]
    in_maps = []
    for c in range(N_CORES):
        xs = xb16[c * NS:(c + 1) * NS]
        # [m2, j, h, ko, n] from x[(m2*2+h)*128+n, ko*128+j]
        xbl = np.ascontiguousarray(
            xs.reshape(MT2, 2, P, KO, P).transpose(0, 4, 1, 3, 2))
        x8s = x8_all[c * NS:(c + 1) * NS]
        x8l = np.ascontiguousarray(
            x8s.reshape(MT2, 2, P, 2, P).transpose(0, 4, 1, 3, 2))
        in_maps.append({"xb": xbl, "x8": x8l, "wt": wt, "w8": w8,
                        "bias": bias128})
    return in_maps


def kernel(x, weight, bias, rewire_rows, rewire_src, rewire_clones):
    in_maps = _pack_inputs(x, weight, bias, rewire_rows, rewire_src,
                           rewire_clones)
    nc = _build_nc()
    res = run_bass_kernel_spmd(nc, in_maps, list(range(N_CORES)))
    out = np.concatenate([np.asarray(res.results[c]["out"])
                          for c in range(N_CORES)], axis=0)
    return np.ascontiguousarray(out.astype(np.float32))


# revision 5
# speedup vs baseline: 1.0171x; 1.0171x over previous
"""Trainium2 Bass kernel v3: mixed fp16/fp8 rewired-linear GEMM.

Math: out = x @ W'.T + bias with the rewire correction folded exactly into
W' on the host (see kernel.py).  Data-parallel over 8 cores (4096 rows each).

Speed trick: a fraction of the K reduction runs as fp8e4m3 DoubleRow matmuls
at 2x the fp16 PE rate.  For m-tiles with local index m < MT_FP8, k-subtiles
6..7 (columns 768:1024) use one DoubleRow fp8 matmul instead of two fp16
matmuls.  Both precisions accumulate into the same PSUM chain: ALL operands
are pre-scaled by powers of two (x*16, W'*256 -- exact in fp16, optimal
range for e4m3) so every product carries the same 4096x scale, removed
during the fused PSUM evacuation  out = ps * 2^-12 + bias  on the DVE.

Accuracy: fp8 quantization error contributes 3.7e-2 * sqrt(f_eff) where
f_eff = 0.25 * MT_FP8/32.  MT_FP8=28 -> rel err 1.73e-2 (gate is 2e-2),
verified bit-deterministically against the fixed-seed reference inputs.
"""

import sys
import types

import numpy as np
import ml_dtypes

import concourse.bass as bass  # noqa: F401
import concourse.tile as tile
import concourse.mybir as mybir
from concourse import bacc
from concourse.bass_utils import run_bass_kernel_spmd


def _ensure_ntff_hook():
    try:
        import antenv.axon_hooks  # noqa: F401
        return
    except ImportError:
        pass
    mod = types.ModuleType("antenv.axon_hooks")
    _hook = [None]
    mod.set_axon_ntff_profile_hook = lambda h: _hook.__setitem__(0, h)
    mod.get_axon_ntff_profile_hook = lambda: _hook[0]
    sys.modules["antenv.axon_hooks"] = mod
    try:
        import antenv
        antenv.axon_hooks = mod
        from trn_agent_boot.trn_boot import _ntff_profile_via_ctypes
        mod.set_axon_ntff_profile_hook(
            _ntff_profile_via_ctypes('/opt/axon/libaxon_pjrt.so'))
    except Exception:
        pass


_ensure_ntff_hook()

N_CORES = 8
N = 32768
IN_F = 1024
OUT_F = 1024
P = 128
NS = N // N_CORES          # 4096 rows per core
MT = NS // P               # 32 m-tiles per core
MT2 = MT // 2              # 16 double-m-tiles
KO = IN_F // P             # 8 k-subtiles
KO16 = KO - 2              # k-subtiles 0..5 always fp16
OC = 512                   # PSUM free-dim chunk (one fp32 bank)

MT_FP8 = 28                # m-tiles (of 32 per core) using fp8 for k 6..7

X_SCALE = 16.0
W_SCALE = 256.0
OUT_SCALE = 1.0 / (X_SCALE * W_SCALE)

_nc_cache = None


def _build_nc():
    global _nc_cache
    if _nc_cache is not None:
        return _nc_cache

    nc = bacc.Bacc("TRN2", target_bir_lowering=False, debug=False)
    xb_d = nc.dram_tensor("xb", [MT2, P, 2, KO, P], mybir.dt.float16,
                          kind="ExternalInput")
    x8_d = nc.dram_tensor("x8", [MT2, P, 2, 2, P], mybir.dt.float8e4,
                          kind="ExternalInput")
    wt_d = nc.dram_tensor("wt", [KO, P, OUT_F], mybir.dt.float16,
                          kind="ExternalInput")
    w8_d = nc.dram_tensor("w8", [P, 2, OUT_F], mybir.dt.float8e4,
                          kind="ExternalInput")
    bias_d = nc.dram_tensor("bias", [P, OUT_F], mybir.dt.float16,
                            kind="ExternalInput")
    out_d = nc.dram_tensor("out", [NS, OUT_F], mybir.dt.float16,
                           kind="ExternalOutput")

    with tile.TileContext(nc) as tc:
        with (
            tc.tile_pool(name="wpool", bufs=1) as wpool,
            tc.tile_pool(name="xpool", bufs=2) as xpool,
            tc.tile_pool(name="x8pool", bufs=2) as x8pool,
            tc.tile_pool(name="opool", bufs=4) as opool,
            tc.tile_pool(name="pspool", bufs=7, space="PSUM") as pspool,
            tc.tile_pool(name="warmpool", bufs=1) as warmpool,
            tc.tile_pool(name="warmps", bufs=1, space="PSUM") as warmps,
        ):
            # Prelude DMA schedule, ALL on the Sync queue in strict need
            # order (the two hwdge queues share the same 16 DMA engines /
            # ~330GB/s, so splitting only reorders, never adds bandwidth).
            # fp8 m-tiles don't touch W tiles 6..7, so those and bias are
            # deferred (emitted inside the loop so their SP program position
            # follows xt[1]); they arrive long before they're needed.
            xt0 = xpool.tile([P, 2, KO, P], mybir.dt.float16, tag="xt")
            nc.sync.dma_start(xt0[:, 0], xb_d.ap()[0, :, 0])
            w01 = wpool.tile([P, 2, OUT_F], mybir.dt.float16, tag="w01")
            nc.sync.dma_start(w01[:], wt_d.ap()[0:2].transpose([1, 0, 2]))
            w23 = wpool.tile([P, 2, OUT_F], mybir.dt.float16, tag="w23")
            nc.sync.dma_start(w23[:], wt_d.ap()[2:4].transpose([1, 0, 2]))
            w45 = wpool.tile([P, 2, OUT_F], mybir.dt.float16, tag="w45")
            nc.sync.dma_start(w45[:], wt_d.ap()[4:6].transpose([1, 0, 2]))
            x8t0 = x8pool.tile([P, 2, 2, P], mybir.dt.float8e4, tag="x8t")
            nc.sync.dma_start(x8t0[:], x8_d.ap()[0])
            w8_sb = wpool.tile([P, 2, OUT_F], mybir.dt.float8e4, tag="w8")
            nc.sync.dma_start(w8_sb[:], w8_d.ap())
            nc.sync.dma_start(xt0[:, 1], xb_d.ap()[0, :, 1])
            # Software-pipeline the second super-tile and the late-needed
            # tiles here so emission order == dependency order (bias is
            # first read at m=0's evacuation, w67 first at m-tile MT_FP8;
            # both trail xt[1] in the Sync queue but arrive in time).
            xt1 = xpool.tile([P, 2, KO, P], mybir.dt.float16, tag="xt")
            nc.sync.dma_start(xt1[:], xb_d.ap()[1])
            x8t1 = x8pool.tile([P, 2, 2, P], mybir.dt.float8e4, tag="x8t")
            nc.sync.dma_start(x8t1[:], x8_d.ap()[1])
            bias_sb = wpool.tile([P, OUT_F], mybir.dt.float16, tag="bias")
            nc.sync.dma_start(bias_sb[:], bias_d.ap())
            w67 = wpool.tile([P, 2, OUT_F], mybir.dt.float16, tag="w67")
            nc.sync.dma_start(w67[:], wt_d.ap()[6:8].transpose([1, 0, 2]))

            def wslice(ko, osl):
                if ko < 2:
                    return w01[:, ko, osl]
                if ko < 4:
                    return w23[:, ko - 2, osl]
                if ko < 6:
                    return w45[:, ko - 4, osl]
                return w67[:, ko - 6, osl]

            # PE warm-up on a zeroed tile while the input DMAs stream in
            # (clock ramp: 0.65 -> 2.4 GHz needs ~3us of PE activity); sized
            # to end right as the first m-tile's operands land.
            wrm = warmpool.tile([P, P], mybir.dt.float16, tag="wrm")
            nc.gpsimd.memset(wrm[:], 0.0)
            wps = warmps.tile([P, P], mybir.dt.float32, tag="wps")
            for _ in range(40):
                nc.tensor.matmul(wps[:], wrm[:], wrm[:], start=True, stop=True)

            NOC = OUT_F // OC
            for m2 in range(MT2):
                if m2 == 0:
                    xt, x8t = xt0, x8t0
                elif m2 == 1:
                    xt, x8t = xt1, x8t1
                else:
                    xt = xpool.tile([P, 2, KO, P], mybir.dt.float16, tag="xt")
                    nc.sync.dma_start(xt[:], xb_d.ap()[m2])
                    x8t = x8pool.tile([P, 2, 2, P], mybir.dt.float8e4,
                                      tag="x8t")
                    nc.sync.dma_start(x8t[:], x8_d.ap()[m2])

                for h in range(2):
                    m = 2 * m2 + h
                    use8 = m < MT_FP8
                    out_sb = opool.tile([P, OUT_F], mybir.dt.float16,
                                        tag="osb")
                    pss = [pspool.tile([P, OC], mybir.dt.float32, tag="ps",
                                       name=f"ps{m}_{oc}")
                           for oc in range(NOC)]
                    nko16 = KO16 if use8 else KO
                    for ko in range(nko16):
                        for oc in range(NOC):
                            nc.tensor.matmul(
                                pss[oc][:],
                                xt[:, h, ko, :],
                                wslice(ko, slice(oc * OC, (oc + 1) * OC)),
                                start=(ko == 0),
                                stop=(not use8 and ko == KO - 1),
                            )
                    if use8:
                        for oc in range(NOC):
                            nc.tensor.matmul(
                                pss[oc][:],
                                x8t[:, h, :, :],
                                w8_sb[:, :, oc * OC:(oc + 1) * OC],
                                start=False,
                                stop=True,
                                perf_mode=mybir.MatmulPerfMode.DoubleRow,
                            )
                    for oc in range(NOC):
                        nc.vector.scalar_tensor_tensor(
                            out_sb[:, oc * OC:(oc + 1) * OC], pss[oc][:],
                            OUT_SCALE,
                            bias_sb[:, oc * OC:(oc + 1) * OC],
                            mybir.AluOpType.mult, mybir.AluOpType.add,
                        )
                        if m >= MT - 4:
                            # Tail: store each PSUM half as soon as it's
                            # evacuated, alternating DMA queues.
                            eng = nc.scalar if (m * NOC + oc) % 2 else nc.sync
                            eng.dma_start(
                                out_d.ap()[m * P:(m + 1) * P,
                                           oc * OC:(oc + 1) * OC],
                                out_sb[:, oc * OC:(oc + 1) * OC])
                    if m < MT - 4:
                        nc.scalar.dma_start(out_d.ap()[m * P:(m + 1) * P, :],
                                            out_sb[:])

    nc.compile()
    _nc_cache = nc
    return nc


def _fold_rewires(weight, rewire_rows, rewire_src, rewire_clones):
    """Fold the rewire corrections into the weight matrix (exact, fp32)."""
    r = np.asarray(rewire_rows, dtype=np.int64)
    s = np.asarray(rewire_src, dtype=np.int64)
    d = np.asarray(rewire_clones, dtype=np.int64)
    denom = d.shape[1] + 1
    w_rs = weight[r, s]                      # [R]
    w_rd = weight[r[:, None], d]             # [R, K]
    dW = np.zeros_like(weight)
    np.add.at(dW, (r, s), (1.0 / denom - 1.0) * w_rs + w_rd.sum(axis=1) / denom)
    np.add.at(dW, (r[:, None], d), -w_rd)
    return weight + dW


def _pack_inputs(x, weight, bias, rewire_rows, rewire_src, rewire_clones):
    e4 = ml_dtypes.float8_e4m3
    wp = _fold_rewires(np.asarray(weight, dtype=np.float32),
                       rewire_rows, rewire_src, rewire_clones)
    wps = np.ascontiguousarray(wp.T) * W_SCALE   # [k_g, o], scaled
    wt = wps.astype(np.float16).reshape(KO, P, OUT_F)
    # fp8 W for k-subtiles 6..7: [k, i, o]
    w8 = np.ascontiguousarray(
        wps[KO16 * P:].astype(e4).reshape(2, P, OUT_F).transpose(1, 0, 2))
    bias128 = np.ascontiguousarray(
        np.broadcast_to(np.asarray(bias, dtype=np.float32).astype(np.float16),
                        (P, OUT_F)))

    xs_all = np.asarray(x, dtype=np.float32) * X_SCALE
    xb16 = xs_all.astype(np.float16)
    x8_all = xs_all[:, KO16 * P:].astype(e4)     # [N, 256]
    in_maps = []
    for c in range(N_CORES):
        xs = xb16[c * NS:(c + 1) * NS]
        # [m2, j, h, ko, n] from x[(m2*2+h)*128+n, ko*128+j]
        xbl = np.ascontiguousarray(
            xs.reshape(MT2, 2, P, KO, P).transpose(0, 4, 1, 3, 2))
        x8s = x8_all[c * NS:(c + 1) * NS]
        x8l = np.ascontiguousarray(
            x8s.reshape(MT2, 2, P, 2, P).transpose(0, 4, 1, 3, 2))
        in_maps.append({"xb": xbl, "x8": x8l, "wt": wt, "w8": w8,
                        "bias": bias128})
    return in_maps


def kernel(x, weight, bias, rewire_rows, rewire_src, rewire_clones):
    in_maps = _pack_inputs(x, weight, bias, rewire_rows, rewire_src,
                           rewire_clones)
    nc = _build_nc()
    res = run_bass_kernel_spmd(nc, in_maps, list(range(N_CORES)))
    out = np.concatenate([np.asarray(res.results[c]["out"])
                          for c in range(N_CORES)], axis=0)
    return np.ascontiguousarray(out.astype(np.float32))
